# revision 21
# baseline (speedup 1.0000x reference)
"""Trainium2 Bass kernel: 3D-window sparse multi-head attention.

Full op: out = SDPA(hid@Wq, hid@Wk, hid@Wv; 3D local window mask) @ Wo + bo
Shapes: hid [1, 2048, 1024], 16 heads x 64, grid (8 frames, 16, 16), window (3, 5, 5).

Sharding: head-parallel. Each of the 8 cores computes 2 heads end-to-end
(QKV projection slices, windowed attention, Wo row-slice projection) and
writes a full-shape fp32 partial; the host sums the 8 partials and adds bo.

Per-core schedule (single NEFF, fully unrolled, Tile-scheduled):
  phase 1: q,k projections interleaved kc-outer (PE saturates while hidT
           streams in), then v in [s, hd] layout with an appended ones
           column for the softmax denominator.
  phase 2: per frame f: for each head, scoresT blocks [kv=128, q=256]
           (kv band = frames f-1..f+1), exp on ACT (no max-subtraction;
           scores are O(5)), multiplicative (h,w) window mask on DVE,
           PV matmul accumulating [65, 256] (row 64 = denominator),
           reciprocal straight from PSUM, normalization via a broadcast
           matmul + PSUM*PSUM multiply, then this frame's two s-chunks of
           the output projection and their DMA — so out-DMA streams
           throughout instead of serializing at the tail.
"""

import numpy as np

import concourse.bass as bass
import concourse.mybir as mybir
import concourse.tile as tile
from concourse import bacc
from concourse.bass import ds, ts
from concourse.bass_utils import run_bass_kernel_spmd

S, D, NH, HD = 2048, 1024, 16, 64
NCORES = 8
HPC = NH // NCORES          # heads per core = 2
F, GH, GW = 8, 16, 16       # frames, height, width (S = F*GH*GW)
WF, WH, WW = 3, 5, 5        # window sizes
T = GH * GW                 # tokens per frame = 256
P = 128
KC = D // P                 # 8 contraction chunks
SC = S // P                 # 16 seq chunks of 128
NQ = S // 512               # 4 free chunks of 512
VW = 2 * 66                 # v1 row width: [h0 64 | ones 2 | h1 64 | ones 2]
LQ = 160                    # live q columns per kv chunk (h-band of the mask)
F16 = mybir.dt.float16
F32 = mybir.dt.float32
EXP = mybir.ActivationFunctionType.Exp
MUL = mybir.AluOpType.mult

_nc_cache = {}


def build_nc(debug=False):
    key = bool(debug)
    if key in _nc_cache:
        return _nc_cache[key]
    nc = bacc.Bacc(None, target_bir_lowering=False, debug=False)

    hidt = nc.dram_tensor("hidt", [P, KC, S], F16, kind="ExternalInput")
    wq = nc.dram_tensor("wq", [P, KC, P], F16, kind="ExternalInput")
    wk = nc.dram_tensor("wk", [P, KC, P], F16, kind="ExternalInput")
    wv = nc.dram_tensor("wv", [P, KC, P], F16, kind="ExternalInput")
    wo = nc.dram_tensor("wo", [P, D], F16, kind="ExternalInput")
    m01 = nc.dram_tensor("m01", [P, 2, LQ], F16, kind="ExternalInput")
    out = nc.dram_tensor("out", [SC, P, D], F32, kind="ExternalOutput")
    dbg = {}
    if debug:
        dbg["qT"] = nc.dram_tensor("dbg_qt", [P, S], F16, kind="ExternalOutput")
        dbg["kT"] = nc.dram_tensor("dbg_kt", [P, S], F16, kind="ExternalOutput")
        dbg["v1"] = nc.dram_tensor("dbg_v1", [P, SC, VW], F16, kind="ExternalOutput")
        dbg["rp"] = nc.dram_tensor("dbg_rp", [1, 2 * S], F16, kind="ExternalOutput")
        dbg["oTn"] = nc.dram_tensor("dbg_otn", [P, S], F16, kind="ExternalOutput")

    with tile.TileContext(nc) as tc:
        with (
            tc.tile_pool(name="const", bufs=1) as cpool,
            tc.tile_pool(name="qk", bufs=1) as qkpool,
            tc.tile_pool(name="attn", bufs=4) as apool,
            tc.tile_pool(name="acc", bufs=1) as accpool,
            tc.tile_pool(name="ostage", bufs=4) as opool,
        ):
            # ---- constant loads: small weights first so PE starts ASAP ----
            wq_sb = cpool.tile([P, KC, P], F16, tag="wq")
            nc.sync.dma_start(wq_sb[:], wq[:])
            wk_sb = cpool.tile([P, KC, P], F16, tag="wk")
            nc.sync.dma_start(wk_sb[:], wk[:])
            wv_sb = cpool.tile([P, KC, P], F16, tag="wv")
            nc.sync.dma_start(wv_sb[:], wv[:])
            wo_sb = cpool.tile([P, D], F16, tag="wo")
            nc.sync.dma_start(wo_sb[:], wo[:])
            m01_sb = cpool.tile([P, 2, LQ], F16, tag="m01")
            nc.sync.dma_start(m01_sb[:], m01[:])
            hidT_sb = cpool.tile([P, KC, S], F16, tag="hidT")
            for kc in range(KC):
                nc.sync.dma_start(hidT_sb[:, kc, :], hidt[:, kc, :])

            qT_sb = qkpool.tile([P, S], F16, tag="qT")
            kT_sb = qkpool.tile([P, S], F16, tag="kT")
            v1_sb = qkpool.tile([P, SC, VW], F16, tag="v1")
            oTn_sb = accpool.tile([P, S], F16, tag="oTn")
            # reciprocal denominators, head h at cols [h*S, (h+1)*S) of row 0
            # (gpsimd partition_broadcast requires its source on partition 0)
            rp_sb = accpool.tile([1, HPC * S], F16, tag="rp")

            # ones columns for the PV denominator row
            nc.vector.memset(v1_sb[:, :, HD : HD + 2], 1.0)
            nc.vector.memset(v1_sb[:, :, 66 + HD : 66 + HD + 2], 1.0)

            # ---- phase 1a: q,k projections (kc-outer: overlap hidT DMA) ----
            with tc.tile_pool(name="pqk", bufs=8, space="PSUM") as pqk:
                psqs = [pqk.tile([P, 512], F32, tag="pqk", name=f"psq{i}")
                        for i in range(NQ)]
                psks = [pqk.tile([P, 512], F32, tag="pqk", name=f"psk{i}")
                        for i in range(NQ)]
                for kc in range(KC):
                    for nch in range(NQ):
                        nc.tensor.matmul(
                            psqs[nch][:], wq_sb[:, kc, :],
                            hidT_sb[:, kc, ts(nch, 512)],
                            start=(kc == 0), stop=(kc == KC - 1),
                        )
                        nc.tensor.matmul(
                            psks[nch][:], wk_sb[:, kc, :],
                            hidT_sb[:, kc, ts(nch, 512)],
                            start=(kc == 0), stop=(kc == KC - 1),
                        )
                for nch in range(NQ):
                    nc.scalar.copy(qT_sb[:, ts(nch, 512)], psqs[nch][:])
                    nc.vector.tensor_copy(kT_sb[:, ts(nch, 512)], psks[nch][:])

            # ---- phase 2: per-frame attention + normalize + O-proj + DMA,
            # with v-projection s-chunks interleaved as PE filler while
            # DVE/ACT chew on exp/mask/copies ----
            with (
                tc.tile_pool(name="pv", bufs=2, space="PSUM") as pvp,
                tc.tile_pool(name="psS", bufs=3, space="PSUM") as pssp,
                tc.tile_pool(name="pso", bufs=1, space="PSUM") as psop,
                tc.tile_pool(name="pO", bufs=2, space="PSUM") as pOp,
                nc.allow_low_precision("softmax reciprocal in fp16"),
            ):
                def v_chunk(sc):
                    psv = pvp.tile([P, P], F32, tag="psv")
                    for kc in range(KC):
                        nc.tensor.matmul(
                            psv[:], hidT_sb[:, kc, ts(sc, P)], wv_sb[:, kc, :],
                            start=(kc == 0), stop=(kc == KC - 1),
                        )
                    # one strided copy: psv [p, 2, 64] -> v1 cols {0:64, 66:130}
                    dst = v1_sb[:, sc, :].rearrange(
                        "p (two c) -> p two c", two=2
                    )[:, :, 0:HD]
                    src = psv[:].rearrange("p (two c) -> p two c", two=2)
                    nc.vector.tensor_copy(dst, src)

                for sc in range(4):          # frame 0 needs kv s-chunks 0..3
                    v_chunk(sc)

                for f in range(F):
                    lo, hi = max(0, f - 1), min(F - 1, f + 1)
                    nb = hi - lo + 1                     # 2-chunk batches
                    fs = ds(f * T, T)
                    # v s-chunks needed by frame f+1 (PE filler work)
                    for sc in (2 * f + 4, 2 * f + 5):
                        if sc < SC:
                            v_chunk(sc)
                    pms = {}
                    # scores + exp + mask for both heads (pipelines on psS).
                    # The (h,w) window implies a kv-h band: an even kv chunk
                    # (kv h 0..7) only reaches q columns 0:160, an odd chunk
                    # (kv h 8..15) only 96:256. Tiles hold just those 160
                    # live columns (m01 is host-packed the same way); PSUM's
                    # per-element has_written bits make the partial-coverage
                    # PV accumulation exact.
                    for h in range(HPC):
                        hr = ds(h * HD, HD)
                        for b in range(nb):
                            psS = pssp.tile([P, 2, LQ], F32, tag="psS")
                            for j in range(2):
                                sckv = 2 * lo + 2 * b + j
                                nc.tensor.matmul(
                                    psS[:, j, :],
                                    kT_sb[hr, ds(sckv * P, P)],
                                    qT_sb[hr, ds(f * T + 96 * j, LQ)],
                                    start=True, stop=True,
                                )
                            et = apool.tile([P, 2, LQ], F16, tag="et")
                            nc.scalar.activation(et[:], psS[:], EXP)
                            pm = apool.tile([P, 2, LQ], F16, tag="pm", bufs=6)
                            nc.vector.tensor_tensor(pm[:], et[:], m01_sb[:], MUL)
                            pms[(h, b)] = pm
                    # PV + normalize per head
                    for h in range(HPC):
                        pso = psop.tile([66, T], F32, tag="pso")
                        for b in range(nb):
                            pm = pms[(h, b)]
                            for j in range(2):
                                sckv = 2 * lo + 2 * b + j
                                nc.tensor.matmul(
                                    pso[0 : HD + 1, ds(96 * j, LQ)],
                                    v1_sb[:, sckv, h * 66 : h * 66 + HD + 1],
                                    pm[:, j, :],
                                    start=(b == 0 and j == 0),
                                    stop=(b == nb - 1 and j == 1),
                                )
                        # reciprocal of the denominator row, then broadcast it
                        # across 64 partitions on the (otherwise idle) GPSIMD
                        rps = ds(h * S + f * T, T)
                        nc.vector.reciprocal(rp_sb[0:1, rps], pso[HD : HD + 1, :])
                        pbs = apool.tile([HD, T], F16, tag="pbs", bufs=2)
                        nc.gpsimd.partition_broadcast(pbs[:], rp_sb[0:1, rps])
                        nc.vector.tensor_tensor(
                            oTn_sb[ds(h * HD, HD), fs], pso[0:HD, :], pbs[:], MUL
                        )
                    # output projection for this frame's two s-chunks
                    for j in range(2):
                        sc = 2 * f + j
                        for n2 in range(2):
                            pO = pOp.tile([P, 512], F32, tag="pO")
                            nc.tensor.matmul(
                                pO[:], oTn_sb[:, ts(sc, P)],
                                wo_sb[:, ts(n2, 512)],
                                start=True, stop=True,
                            )
                            ob = opool.tile([P, 512], F32, tag="ob")
                            nc.scalar.copy(ob[:], pO[:])
                            nc.sync.dma_start(out[sc, :, ts(n2, 512)], ob[:])

            if debug:
                nc.sync.dma_start(dbg["qT"][:], qT_sb[:])
                nc.sync.dma_start(dbg["kT"][:], kT_sb[:])
                nc.sync.dma_start(dbg["v1"][:], v1_sb[:])
                nc.sync.dma_start(dbg["rp"][:], rp_sb[:])
                nc.sync.dma_start(dbg["oTn"][:], oTn_sb[:])

    nc.compile()
    _nc_cache[key] = nc
    return nc


def make_in_maps(hidden_states, Wq, Wk, Wv, Wo):
    """Host-side shard + repack of full inputs into per-core input maps."""
    hid = np.asarray(hidden_states, np.float32).reshape(S, D)
    # hidT packed [ki, ko, s] with d = ko*128 + ki
    hidT_pk = np.ascontiguousarray(
        hid.T.reshape(KC, P, S).transpose(1, 0, 2)
    ).astype(np.float16)

    scale = 1.0 / np.sqrt(HD)
    Wq_s = np.asarray(Wq, np.float32) * scale
    Wk_ = np.asarray(Wk, np.float32)
    Wv_ = np.asarray(Wv, np.float32)
    Wo_ = np.asarray(Wo, np.float32)

    def pack_w(W, c):
        Wc = W[:, c * HPC * HD : (c + 1) * HPC * HD]  # [D, 128]
        return np.ascontiguousarray(
            Wc.reshape(KC, P, HPC * HD).transpose(1, 0, 2)
        ).astype(np.float16)

    # (h, w) window mask, 0/1, [256, 256] (symmetric), packed to the live
    # window layout [p, j, c]: m01_pk[p, j, c] = W01[j*128 + p, 96*j + c]
    idx = np.arange(T)
    hh, ww = idx // GW, idx % GW
    m = (np.abs(hh[:, None] - hh[None, :]) <= WH // 2) & (
        np.abs(ww[:, None] - ww[None, :]) <= WW // 2
    )
    m01_pk = np.empty((P, 2, LQ), np.float16)
    for j in range(2):
        m01_pk[:, j, :] = m[j * P : (j + 1) * P, 96 * j : 96 * j + LQ]

    in_maps = []
    for c in range(NCORES):
        in_maps.append(
            dict(
                hidt=hidT_pk,
                wq=pack_w(Wq_s, c),
                wk=pack_w(Wk_, c),
                wv=pack_w(Wv_, c),
                wo=Wo_[c * HPC * HD : (c + 1) * HPC * HD, :].astype(np.float16),
                m01=m01_pk,
            )
        )
    return in_maps


def kernel(
    hidden_states,
    Wq,
    Wk,
    Wv,
    Wo,
    bo,
    frames=F,
    height=GH,
    width=GW,
    wf=WF,
    wh=WH,
    ww=WW,
):
    assert (int(frames), int(height), int(width)) == (F, GH, GW)
    assert (int(wf), int(wh), int(ww)) == (WF, WH, WW)
    in_maps = make_in_maps(hidden_states, Wq, Wk, Wv, Wo)
    nc = build_nc(debug=False)
    res = run_bass_kernel_spmd(nc, in_maps, core_ids=list(range(NCORES)))
    acc = np.zeros((S, D), np.float32)
    for r in res.results:
        acc += r["out"].reshape(S, D)
    acc += np.asarray(bo, np.float32)[None, :]
    return acc.reshape(1, S, D)


# revision 24
# speedup vs baseline: 2.7886x; 2.7886x over previous
"""Trainium2 Bass kernel: 3D-window sparse multi-head attention.

Full op: out = SDPA(hid@Wq, hid@Wk, hid@Wv; 3D local window mask) @ Wo + bo
Shapes: hid [1, 2048, 1024], 16 heads x 64, grid (8 frames, 16, 16), window (3, 5, 5).

Sharding: head-parallel. Each of the 8 cores computes 2 heads end-to-end
(QKV projection slices, windowed attention, Wo row-slice projection) and
writes a full-shape fp32 partial; the host sums the 8 partials and adds bo.

Per-core schedule (single NEFF, fully unrolled, Tile-scheduled):
  phase 1: q,k projections interleaved kc-outer (PE saturates while hidT
           streams in), then v in [s, hd] layout with an appended ones
           column for the softmax denominator.
  phase 2: per frame f: for each head, scoresT blocks [kv=128, q=256]
           (kv band = frames f-1..f+1), exp on ACT (no max-subtraction;
           scores are O(5)), multiplicative (h,w) window mask on DVE,
           PV matmul accumulating [65, 256] (row 64 = denominator),
           reciprocal straight from PSUM, normalization via a broadcast
           matmul + PSUM*PSUM multiply, then this frame's two s-chunks of
           the output projection and their DMA — so out-DMA streams
           throughout instead of serializing at the tail.
"""

import numpy as np

import concourse.bass as bass
import concourse.mybir as mybir
import concourse.tile as tile
from concourse import bacc
from concourse.bass import ds, ts
from concourse.bass_utils import run_bass_kernel_spmd

S, D, NH, HD = 2048, 1024, 16, 64
NCORES = 8
HPC = NH // NCORES          # heads per core = 2
F, GH, GW = 8, 16, 16       # frames, height, width (S = F*GH*GW)
WF, WH, WW = 3, 5, 5        # window sizes
T = GH * GW                 # tokens per frame = 256
P = 128
KC = D // P                 # 8 contraction chunks
SC = S // P                 # 16 seq chunks of 128
NQ = S // 512               # 4 free chunks of 512
VW = 2 * 66                 # v1 row width: [h0 64 | ones 2 | h1 64 | ones 2]
LQ = 160                    # live q columns per kv chunk (h-band of the mask)
F16 = mybir.dt.float16
F32 = mybir.dt.float32
EXP = mybir.ActivationFunctionType.Exp
MUL = mybir.AluOpType.mult

_nc_cache = {}


def build_nc(debug=False):
    key = bool(debug)
    if key in _nc_cache:
        return _nc_cache[key]
    nc = bacc.Bacc(None, target_bir_lowering=False, debug=False)

    hidt = nc.dram_tensor("hidt", [P, KC, S], F16, kind="ExternalInput")
    wq = nc.dram_tensor("wq", [P, KC, P], F16, kind="ExternalInput")
    wk = nc.dram_tensor("wk", [P, KC, P], F16, kind="ExternalInput")
    wv = nc.dram_tensor("wv", [P, KC, P], F16, kind="ExternalInput")
    wo = nc.dram_tensor("wo", [P, D], F16, kind="ExternalInput")
    m01 = nc.dram_tensor("m01", [P, 2, LQ], F16, kind="ExternalInput")
    out = nc.dram_tensor("out", [SC, P, D], F32, kind="ExternalOutput")
    dbg = {}
    if debug:
        dbg["qT"] = nc.dram_tensor("dbg_qt", [P, S], F16, kind="ExternalOutput")
        dbg["kT"] = nc.dram_tensor("dbg_kt", [P, S], F16, kind="ExternalOutput")
        dbg["v1"] = nc.dram_tensor("dbg_v1", [P, SC, VW], F16, kind="ExternalOutput")
        dbg["rp"] = nc.dram_tensor("dbg_rp", [1, 2 * S], F16, kind="ExternalOutput")
        dbg["oTn"] = nc.dram_tensor("dbg_otn", [P, S], F16, kind="ExternalOutput")

    with tile.TileContext(nc) as tc:
        with (
            tc.tile_pool(name="const", bufs=1) as cpool,
            tc.tile_pool(name="qk", bufs=1) as qkpool,
            tc.tile_pool(name="attn", bufs=4) as apool,
            tc.tile_pool(name="acc", bufs=1) as accpool,
            tc.tile_pool(name="ostage", bufs=4) as opool,
        ):
            # ---- constant loads: small weights first so PE starts ASAP ----
            wq_sb = cpool.tile([P, KC, P], F16, tag="wq")
            nc.sync.dma_start(wq_sb[:], wq[:])
            wk_sb = cpool.tile([P, KC, P], F16, tag="wk")
            nc.sync.dma_start(wk_sb[:], wk[:])
            wv_sb = cpool.tile([P, KC, P], F16, tag="wv")
            nc.sync.dma_start(wv_sb[:], wv[:])
            wo_sb = cpool.tile([P, D], F16, tag="wo")
            nc.sync.dma_start(wo_sb[:], wo[:])
            m01_sb = cpool.tile([P, 2, LQ], F16, tag="m01")
            nc.sync.dma_start(m01_sb[:], m01[:])
            # hidT streamed in 512-column blocks: block nch unlocks the
            # nch-th q/k projection chunk and v s-chunks 4*nch..4*nch+3
            hidT_sb = cpool.tile([P, KC, S], F16, tag="hidT")
            for nch in range(NQ):
                nc.sync.dma_start(
                    hidT_sb[:, :, ts(nch, 512)], hidt[:, :, ts(nch, 512)]
                )

            qT_sb = qkpool.tile([P, S], F16, tag="qT")
            kT_sb = qkpool.tile([P, S], F16, tag="kT")
            v1_sb = qkpool.tile([P, SC, VW], F16, tag="v1")
            oTn_sb = accpool.tile([P, S], F16, tag="oTn")
            # reciprocal denominators, head h at cols [h*S, (h+1)*S) of row 0
            # (gpsimd partition_broadcast requires its source on partition 0)
            rp_sb = accpool.tile([1, HPC * S], F16, tag="rp")

            # ones columns for the PV denominator row
            nc.vector.memset(v1_sb[:, :, HD : HD + 2], 1.0)
            nc.vector.memset(v1_sb[:, :, 66 + HD : 66 + HD + 2], 1.0)

            # ---- interleaved projections + per-frame attention pipeline ----
            with (
                tc.tile_pool(name="pqk", bufs=2, space="PSUM") as pqk,
                tc.tile_pool(name="pv", bufs=1, space="PSUM") as pvp,
                tc.tile_pool(name="psS", bufs=2, space="PSUM") as pssp,
                tc.tile_pool(name="pso", bufs=1, space="PSUM") as psop,
                tc.tile_pool(name="pO", bufs=2, space="PSUM") as pOp,
                nc.allow_low_precision("softmax reciprocal in fp16"),
            ):
                def qk_chunk(nch):
                    psq = pqk.tile([P, 512], F32, tag="pqk", name="psq")
                    for kc in range(KC):
                        nc.tensor.matmul(
                            psq[:], wq_sb[:, kc, :],
                            hidT_sb[:, kc, ts(nch, 512)],
                            start=(kc == 0), stop=(kc == KC - 1),
                        )
                    nc.scalar.copy(qT_sb[:, ts(nch, 512)], psq[:])
                    psk = pqk.tile([P, 512], F32, tag="pqk", name="psk")
                    for kc in range(KC):
                        nc.tensor.matmul(
                            psk[:], wk_sb[:, kc, :],
                            hidT_sb[:, kc, ts(nch, 512)],
                            start=(kc == 0), stop=(kc == KC - 1),
                        )
                    nc.vector.tensor_copy(kT_sb[:, ts(nch, 512)], psk[:])

                def v_chunk(sc):
                    psv = pvp.tile([P, P], F32, tag="psv")
                    for kc in range(KC):
                        nc.tensor.matmul(
                            psv[:], hidT_sb[:, kc, ts(sc, P)], wv_sb[:, kc, :],
                            start=(kc == 0), stop=(kc == KC - 1),
                        )
                    # one strided copy: psv [p, 2, 64] -> v1 cols {0:64, 66:130}
                    dst = v1_sb[:, sc, :].rearrange(
                        "p (two c) -> p two c", two=2
                    )[:, :, 0:HD]
                    src = psv[:].rearrange("p (two c) -> p two c", two=2)
                    nc.vector.tensor_copy(dst, src)

                def o_proj(f):
                    for j in range(2):
                        sc = 2 * f + j
                        for n2 in range(2):
                            pO = pOp.tile([P, 512], F32, tag="pO")
                            nc.tensor.matmul(
                                pO[:], oTn_sb[:, ts(sc, P)],
                                wo_sb[:, ts(n2, 512)],
                                start=True, stop=True,
                            )
                            ob = opool.tile([P, 512], F32, tag="ob")
                            if n2 == 0:
                                nc.vector.tensor_copy(ob[:], pO[:])
                            else:
                                nc.scalar.copy(ob[:], pO[:])
                            nc.sync.dma_start(out[sc, :, ts(n2, 512)], ob[:])

                # PE work interleave: projection chunks land just before the
                # first frame that needs them; each frame's output projection
                # is emitted one frame late so PE never waits the normalize
                # chain (recip -> broadcast -> multiply).
                pre = {
                    0: ["qk0", "v0", "v1", "v2", "v3"],
                    1: ["qk1", "v4", "v5"],
                    2: ["v6", "v7"],
                    3: ["qk2", "v8", "v9"],
                    4: ["v10", "v11"],
                    5: ["qk3", "v12", "v13"],
                    6: ["v14", "v15"],
                    7: [],
                }
                for f in range(F):
                    for w in pre[f]:
                        if w.startswith("qk"):
                            qk_chunk(int(w[2:]))
                        else:
                            v_chunk(int(w[1:]))
                    lo, hi = max(0, f - 1), min(F - 1, f + 1)
                    nb = hi - lo + 1                     # 2-chunk batches
                    fs = ds(f * T, T)
                    pms = {}
                    # scores + exp + mask for both heads (pipelines on psS).
                    # The (h,w) window implies a kv-h band: an even kv chunk
                    # (kv h 0..7) only reaches q columns 0:160, an odd chunk
                    # (kv h 8..15) only 96:256. Tiles hold just those 160
                    # live columns (m01 is host-packed the same way); PSUM's
                    # per-element has_written bits make the partial-coverage
                    # PV accumulation exact.
                    for h in range(HPC):
                        hr = ds(h * HD, HD)
                        for b in range(nb):
                            psS = pssp.tile([P, 2, LQ], F32, tag="psS")
                            for j in range(2):
                                sckv = 2 * lo + 2 * b + j
                                nc.tensor.matmul(
                                    psS[:, j, :],
                                    kT_sb[hr, ds(sckv * P, P)],
                                    qT_sb[hr, ds(f * T + 96 * j, LQ)],
                                    start=True, stop=True,
                                )
                            et = apool.tile([P, 2, LQ], F16, tag="et")
                            nc.scalar.activation(et[:], psS[:], EXP)
                            pm = apool.tile([P, 2, LQ], F16, tag="pm", bufs=6)
                            nc.vector.tensor_tensor(pm[:], et[:], m01_sb[:], MUL)
                            pms[(h, b)] = pm
                    # PV + normalize per head
                    for h in range(HPC):
                        pso = psop.tile([66, T], F32, tag="pso")
                        for b in range(nb):
                            pm = pms[(h, b)]
                            for j in range(2):
                                sckv = 2 * lo + 2 * b + j
                                nc.tensor.matmul(
                                    pso[0 : HD + 1, ds(96 * j, LQ)],
                                    v1_sb[:, sckv, h * 66 : h * 66 + HD + 1],
                                    pm[:, j, :],
                                    start=(b == 0 and j == 0),
                                    stop=(b == nb - 1 and j == 1),
                                )
                        # reciprocal of the denominator row, then broadcast it
                        # across 64 partitions on the (otherwise idle) GPSIMD
                        rps = ds(h * S + f * T, T)
                        nc.vector.reciprocal(rp_sb[0:1, rps], pso[HD : HD + 1, :])
                        pbs = apool.tile([HD, T], F16, tag="pbs", bufs=2)
                        nc.gpsimd.partition_broadcast(pbs[:], rp_sb[0:1, rps])
                        nc.vector.tensor_tensor(
                            oTn_sb[ds(h * HD, HD), fs], pso[0:HD, :], pbs[:], MUL
                        )
                    # lagged output projection: frame f-1's O-proj runs while
                    # frame f's normalize chain completes
                    if f > 0:
                        o_proj(f - 1)
                o_proj(F - 1)

            if debug:
                nc.sync.dma_start(dbg["qT"][:], qT_sb[:])
                nc.sync.dma_start(dbg["kT"][:], kT_sb[:])
                nc.sync.dma_start(dbg["v1"][:], v1_sb[:])
                nc.sync.dma_start(dbg["rp"][:], rp_sb[:])
                nc.sync.dma_start(dbg["oTn"][:], oTn_sb[:])

    nc.compile()
    _nc_cache[key] = nc
    return nc


def make_in_maps(hidden_states, Wq, Wk, Wv, Wo):
    """Host-side shard + repack of full inputs into per-core input maps."""
    hid = np.asarray(hidden_states, np.float32).reshape(S, D)
    # hidT packed [ki, ko, s] with d = ko*128 + ki
    hidT_pk = np.ascontiguousarray(
        hid.T.reshape(KC, P, S).transpose(1, 0, 2)
    ).astype(np.float16)

    scale = 1.0 / np.sqrt(HD)
    Wq_s = np.asarray(Wq, np.float32) * scale
    Wk_ = np.asarray(Wk, np.float32)
    Wv_ = np.asarray(Wv, np.float32)
    Wo_ = np.asarray(Wo, np.float32)

    def pack_w(W, c):
        Wc = W[:, c * HPC * HD : (c + 1) * HPC * HD]  # [D, 128]
        return np.ascontiguousarray(
            Wc.reshape(KC, P, HPC * HD).transpose(1, 0, 2)
        ).astype(np.float16)

    # (h, w) window mask, 0/1, [256, 256] (symmetric), packed to the live
    # window layout [p, j, c]: m01_pk[p, j, c] = W01[j*128 + p, 96*j + c]
    idx = np.arange(T)
    hh, ww = idx // GW, idx % GW
    m = (np.abs(hh[:, None] - hh[None, :]) <= WH // 2) & (
        np.abs(ww[:, None] - ww[None, :]) <= WW // 2
    )
    m01_pk = np.empty((P, 2, LQ), np.float16)
    for j in range(2):
        m01_pk[:, j, :] = m[j * P : (j + 1) * P, 96 * j : 96 * j + LQ]

    in_maps = []
    for c in range(NCORES):
        in_maps.append(
            dict(
                hidt=hidT_pk,
                wq=pack_w(Wq_s, c),
                wk=pack_w(Wk_, c),
                wv=pack_w(Wv_, c),
                wo=Wo_[c * HPC * HD : (c + 1) * HPC * HD, :].astype(np.float16),
                m01=m01_pk,
            )
        )
    return in_maps


def kernel(
    hidden_states,
    Wq,
    Wk,
    Wv,
    Wo,
    bo,
    frames=F,
    height=GH,
    width=GW,
    wf=WF,
    wh=WH,
    ww=WW,
):
    assert (int(frames), int(height), int(width)) == (F, GH, GW)
    assert (int(wf), int(wh), int(ww)) == (WF, WH, WW)
    in_maps = make_in_maps(hidden_states, Wq, Wk, Wv, Wo)
    nc = build_nc(debug=False)
    res = run_bass_kernel_spmd(nc, in_maps, core_ids=list(range(NCORES)))
    acc = np.zeros((S, D), np.float32)
    for r in res.results:
        acc += r["out"].reshape(S, D)
    acc += np.asarray(bo, np.float32)[None, :]
    return acc.reshape(1, S, D)
